# revision 1
# baseline (speedup 1.0000x reference)
"""Trainium2 Bass kernel for DocREModel_KD head (ragged_sequence).

Problem shape (hardcoded, per spec):
  sequence_output [4, 1024, 768] f32
  attention       [4, 12, 1024, 1024] f32
  entity_starts   [4, 42, 4] int
  hts             [4, 1764, 2] int
Outputs: (hss, rss, tss) each [4, 42, 42, 768] f32.

Strategy (8 cores, SPMD single program):
  - 2 cores per document, split by entity pair. The pair grid G over the
    42x42 entity pairs is symmetric, so each core computes only canonical
    (min<=max) representatives: 3 padded i-blocks of 7 rows, block b
    covering j in [7b, 42), 735 packed rows total. Core parity rho=1
    receives entity indices rotated by 21 (host-side permutation of the
    tiny index tensors), so the identical program computes the pairs whose
    min entity lies in the other half. Host maps any (h,t) through the
    symmetric representative via _grid_row_table().
  - All data-dependent gathers use indirect DMA with host-computed row
    indices fed as int32 inputs (SPMD-safe).
  - e_att (mention-mean of attention rows) is computed c-partitioned by a
    single fused PE matmul against a constant selection matrix (mean +
    transpose in one step, f32r).
  - Pair grid G[c,(i,j)] = sum_h EA[c,i,h]*EA[c,j,h] via broadcast-AP DVE
    products (bf16, 2x mode, one instruction per i-block) + grouped tree
    reduction (12->4->2->1), relu on ScalarE.
  - rs = (relu(G) @ seq_aug) with an appended ones column giving the
    normalizer for free; per tau the psB (ones-column) matmuls run first so
    the reciprocal overlaps the psA matmuls; normalization folded into the
    PSUM drains (DVE + ACT in parallel).
  - e_emb logsumexp is d-split across the core pair (rho chooses which half
    of the hidden dim), exp/ln on ScalarE.
  - hss/tss (pure row replications of e_emb) and the hts->grid-row mapping
    are assembled host-side from the device-computed e_emb / rs grid.

Measured: ~160-164 us HW exec on cores 0-7, rel err 4.4e-3 (bf16-dominated,
rss only; hss/tss exact to ~2e-6).
"""

import numpy as np
from contextlib import ExitStack

import concourse.bass as bass
import concourse.bacc as bacc
import concourse.mybir as mybir
import concourse.tile as tile
from concourse.bass_utils import run_bass_kernel_spmd

# ---- problem constants ----
B, H, C, HS, NE, M = 4, 12, 1024, 768, 42, 4
OFFSET = 1
NH = NE * H          # 504 (n,h) pairs
IL = NE // 2         # 21 grid rows per core
NB = 3               # i-blocks of 7 rows; block b covers j in [7b, 42)
BW = 7
BLKW = [NE - BW * b for b in range(NB)]        # 42, 35, 28
BLKOFF = [0, BW * BLKW[0], BW * (BLKW[0] + BLKW[1])]  # row offsets: 0, 294, 539
U = BW * sum(BLKW)   # 735 packed grid rows per core (canonical min<=max reps)
PPT = 126            # partitions per gathered RAW tile (504 = 4*126)
NCH = C // 128       # 8 c-chunks
WLSE = HS // 2       # 384: e_emb d-split width per core
N_CORES = 8

F32 = mybir.dt.float32
F32R = mybir.dt.float32r
BF16 = mybir.dt.bfloat16
I32 = mybir.dt.int32

_prog_cache = {}


def _build_program():
    nc = bacc.Bacc(None)

    att = nc.dram_tensor("att", [H * C, C], F32R, kind="ExternalInput")
    seq = nc.dram_tensor("seq", [C, HS], F32, kind="ExternalInput")
    seq_lse = nc.dram_tensor("seq_lse", [C, WLSE], F32, kind="ExternalInput")
    sel_d = nc.dram_tensor("sel", [PPT, 4 * NH], F32R, kind="ExternalInput")
    idx_att_d = nc.dram_tensor("idx_att", [PPT, 16], I32, kind="ExternalInput")
    idx_seq_d = nc.dram_tensor("idx_seq", [NE, M], I32, kind="ExternalInput")

    rs_out = nc.dram_tensor("rs_out", [U, HS], F32, kind="ExternalOutput")
    eemb_out = nc.dram_tensor("eemb_out", [NE, WLSE], F32, kind="ExternalOutput")

    with tile.TileContext(nc) as tc, ExitStack() as ctx:
        const_p = ctx.enter_context(tc.tile_pool(name="const", bufs=1))
        raw_p = ctx.enter_context(tc.tile_pool(name="raw", bufs=1))
        seqf_p = ctx.enter_context(tc.tile_pool(name="seqf", bufs=1))
        seqb_p = ctx.enter_context(tc.tile_pool(name="seqb", bufs=1))
        ea_p = ctx.enter_context(tc.tile_pool(name="ea", bufs=1))
        pr_p = ctx.enter_context(tc.tile_pool(name="pr", bufs=1))
        t4_p = ctx.enter_context(tc.tile_pool(name="t4", bufs=1))
        t2_p = ctx.enter_context(tc.tile_pool(name="t2", bufs=1))
        g_p = ctx.enter_context(tc.tile_pool(name="g", bufs=1))
        lse_p = ctx.enter_context(tc.tile_pool(name="lse", bufs=1))
        rst_p = ctx.enter_context(tc.tile_pool(name="rst", bufs=2))
        small_p = ctx.enter_context(tc.tile_pool(name="small", bufs=2))

        ea_ps = ctx.enter_context(tc.tile_pool(name="eaps", bufs=2, space="PSUM"))
        rsA_ps = ctx.enter_context(tc.tile_pool(name="rsA", bufs=3, space="PSUM"))
        rsB_ps = ctx.enter_context(tc.tile_pool(name="rsB", bufs=3, space="PSUM"))

        # --- constants / indices to SBUF ---
        ia_sb = const_p.tile([PPT, 16], I32, name="ia_sb")
        nc.sync.dma_start(out=ia_sb[:], in_=idx_att_d[:])
        is_sb = const_p.tile([NE, M], I32, name="is_sb")
        nc.sync.dma_start(out=is_sb[:], in_=idx_seq_d[:])
        sel_sb = const_p.tile([PPT, 4 * NH], F32R, name="sel_sb")
        nc.sync.dma_start(out=sel_sb[:], in_=sel_d[:])

        # --- indirect gathers: attention mention rows ---
        raws = []
        for t in range(16):
            rt = raw_p.tile([PPT, C], F32R, name=f"raw{t}")
            nc.gpsimd.indirect_dma_start(
                out=rt[:],
                out_offset=None,
                in_=att[:],
                in_offset=bass.IndirectOffsetOnAxis(ap=ia_sb[:, t : t + 1], axis=0),
            )
            raws.append(rt)

        # --- e_emb logsumexp pipeline (d-split half, exact fp32) ---
        sg = []
        for r in range(M):
            g = lse_p.tile([NE, WLSE], F32, name=f"sg{r}")
            nc.gpsimd.indirect_dma_start(
                out=g[:],
                out_offset=None,
                in_=seq_lse[:],
                in_offset=bass.IndirectOffsetOnAxis(ap=is_sb[:, r : r + 1], axis=0),
            )
            sg.append(g)
        ex = []
        for r in range(M):
            e = lse_p.tile([NE, WLSE], F32, name=f"ex{r}")
            nc.scalar.activation(out=e[:], in_=sg[r][:], func=mybir.ActivationFunctionType.Exp)
            ex.append(e)
        s01 = lse_p.tile([NE, WLSE], F32, name="s01")
        s23 = lse_p.tile([NE, WLSE], F32, name="s23")
        nc.vector.tensor_add(out=s01[:], in0=ex[0][:], in1=ex[1][:])
        nc.vector.tensor_add(out=s23[:], in0=ex[2][:], in1=ex[3][:])
        nc.vector.tensor_add(out=s01[:], in0=s01[:], in1=s23[:])
        lse_res = lse_p.tile([NE, WLSE], F32, name="lse_res")
        nc.scalar.activation(out=lse_res[:], in_=s01[:], func=mybir.ActivationFunctionType.Ln)
        # ACT-issued DMA: same-engine ordering after the Ln, so the DMA
        # carries only its ring-FIFO wait.
        nc.scalar.dma_start(out=eemb_out[:], in_=lse_res[:])

        # --- EA: mention-mean + transpose via SEL matmul (f32r) ---
        eas = []
        for k in range(NCH):
            ps = ea_ps.tile([128, NH], F32, name="eaps")
            for t in range(16):
                q = t % 4
                nc.tensor.matmul(
                    out=ps[:],
                    lhsT=raws[t][:, k * 128 : (k + 1) * 128],
                    rhs=sel_sb[:, q * NH : (q + 1) * NH],
                    start=(t == 0),
                    stop=(t == 15),
                )
            ea = ea_p.tile([128, NH], BF16, name=f"ea{k}")
            nc.scalar.copy(out=ea[:], in_=ps[:])
            eas.append(ea)

        # --- pair-grid products + grouped h-reduction + relu ---
        gs = []
        for k in range(NCH):
            pr = pr_p.tile([128, U * H], BF16, name="pr")
            ea3 = eas[k][:].rearrange("p (i h) -> p i h", h=H)          # [128, 42, 12]
            for b in range(NB):
                w = BLKW[b]
                jf = BW * b
                in0 = ea3[:, jf : jf + BW, :].unsqueeze(2).to_broadcast([128, BW, w, H])
                in1 = ea3[:, jf:NE, :].unsqueeze(1).to_broadcast([128, BW, w, H])
                sec = pr[:, BLKOFF[b] * H : (BLKOFF[b] + BW * w) * H]
                pr4 = sec.rearrange("p (i j h) -> p i j h", j=w, h=H)
                nc.vector.tensor_tensor(out=pr4, in0=in0, in1=in1, op=mybir.AluOpType.mult)

            pru = pr[:].rearrange("p (u h) -> p u h", h=H)              # [128, U, 12]
            t4 = t4_p.tile([128, U * 4], BF16, name="t4")
            t4v = t4[:].rearrange("p (u f) -> p u f", f=4)
            nc.vector.tensor_tensor(out=t4v, in0=pru[:, :, 0:4], in1=pru[:, :, 4:8], op=mybir.AluOpType.add)
            nc.vector.tensor_tensor(out=t4v, in0=t4v, in1=pru[:, :, 8:12], op=mybir.AluOpType.add)
            t2 = t2_p.tile([128, U * 2], BF16, name="t2")
            t2v = t2[:].rearrange("p (u f) -> p u f", f=2)
            nc.vector.tensor_tensor(out=t2v, in0=t4v[:, :, 0:2], in1=t4v[:, :, 2:4], op=mybir.AluOpType.add)
            gp = g_p.tile([128, U], BF16, name=f"gp{k}")
            a = t2v[:, :, 0:1].squeeze(2)
            b = t2v[:, :, 1:2].squeeze(2)
            nc.vector.tensor_tensor(out=gp[:], in0=a, in1=b, op=mybir.AluOpType.add)
            g_t = g_p.tile([128, U], BF16, name=f"g{k}")
            nc.scalar.activation(out=g_t[:], in_=gp[:], func=mybir.ActivationFunctionType.Relu)
            gs.append(g_t)

        # --- rs matmul + fused normalization drain (drains on ACT so the
        # ACT-issued output DMA and the PSUM-bank-reuse waits stay single) ---
        # --- sequence chunks: load f32, convert to bf16, append ones col ---
        # (issued after the gathers/products in program order so the big seq
        # DMAs don't compete with the latency-critical attention gathers)
        seqb = []
        for k in range(NCH):
            sf = seqf_p.tile([128, HS], F32, name=f"sf{k}")
            nc.sync.dma_start(out=sf[:], in_=seq[k * 128 : (k + 1) * 128, :])
            sb = seqb_p.tile([128, HS + 1], BF16, name=f"sb{k}")
            nc.scalar.copy(out=sb[:, 0:HS], in_=sf[:])
            nc.vector.memset(sb[:, HS : HS + 1], 1.0)
            seqb.append(sb)

        ntau = (U + PPT - 1) // PPT                      # 6 (last tau: 105 rows)
        for tau in range(ntau):
            lo = tau * PPT
            rows = min(PPT, U - lo)
            psA = rsA_ps.tile([PPT, 512], F32, name="psA")
            psB = rsB_ps.tile([PPT, HS + 1 - 512], F32, name="psB")   # [126, 257]
            for k in range(NCH):
                nc.tensor.matmul(
                    out=psB[:rows],
                    lhsT=gs[k][:, lo : lo + rows],
                    rhs=seqb[k][:, 512 : HS + 1],
                    start=(k == 0),
                    stop=(k == NCH - 1),
                )
            dsum = small_p.tile([PPT, 1], F32, name="dsum")
            nc.vector.tensor_scalar_add(out=dsum[:rows], in0=psB[:rows, 256:257], scalar1=1e-10)
            drec = small_p.tile([PPT, 1], F32, name="drec")
            nc.vector.reciprocal(out=drec[:rows], in_=dsum[:rows])
            for k in range(NCH):
                nc.tensor.matmul(
                    out=psA[:rows],
                    lhsT=gs[k][:, lo : lo + rows],
                    rhs=seqb[k][:, 0:512],
                    start=(k == 0),
                    stop=(k == NCH - 1),
                )
            st = rst_p.tile([PPT, HS], F32, name="st")
            nc.vector.tensor_scalar_mul(out=st[:rows, 0:512], in0=psA[:rows], scalar1=drec[:rows])
            nc.scalar.activation(
                out=st[:rows, 512:HS], in_=psB[:rows, 0:256],
                func=mybir.ActivationFunctionType.Copy, scale=drec[:rows],
            )
            nc.sync.dma_start(out=rs_out[lo : lo + rows, :], in_=st[:rows])

    nc.finalize()
    return nc


def _host_inputs(sequence_output, attention, entity_starts):
    """Build the 8 per-core input maps."""
    sel_np = np.zeros([PPT, 4 * NH], np.float32)
    for q in range(4):
        for p in range(PPT):
            sel_np[p, q * NH + q * PPT + p] = 0.25

    in_maps = []
    for cid in range(N_CORES):
        d, rho = cid // 2, cid % 2
        perm = (np.arange(NE) + rho * IL) % NE            # slot -> real entity
        starts_doc = np.asarray(entity_starts[d], dtype=np.int64)
        pstarts = starts_doc[perm]                        # [42, 4]
        pos = pstarts + OFFSET                            # mention positions, < 1024

        ia = np.zeros([PPT, 16], np.int32)
        for t in range(16):
            r, q = t // 4, t % 4
            p = np.arange(PPT)
            g = q * PPT + p
            n, h = g // H, g % H
            ia[:, t] = (h * C + pos[n, r]).astype(np.int32)

        iseq = pos.astype(np.int32)                       # [42, 4]

        att_doc = np.ascontiguousarray(
            np.asarray(attention[d], dtype=np.float32).reshape(H * C, C)
        )
        seq_doc = np.ascontiguousarray(np.asarray(sequence_output[d], dtype=np.float32))
        seq_lse = np.ascontiguousarray(seq_doc[:, rho * WLSE : (rho + 1) * WLSE])

        in_maps.append(
            {
                "att": att_doc,
                "seq": seq_doc,
                "seq_lse": seq_lse,
                "sel": sel_np,
                "idx_att": ia,
                "idx_seq": iseq,
            }
        )
    return in_maps


_row_table_cache = {}


def _grid_row_table():
    if "t" not in _row_table_cache:
        row_of = np.full((IL, NE), -1, np.int64)
        for b in range(NB):
            w = BLKW[b]
            jf = BW * b
            for il in range(BW):
                for j in range(jf, NE):
                    row_of[BW * b + il, j] = BLKOFF[b] + il * w + (j - jf)
        _row_table_cache["t"] = row_of
    return _row_table_cache["t"]


def _assemble(results, entity_starts, hts):
    eemb = np.empty([B, NE, HS], np.float32)
    rs_grid = np.empty([B, NE, NE, HS], np.float32)
    perm1 = (np.arange(NE) + IL) % NE
    for d in range(B):
        o0 = results[2 * d]["eemb_out"]
        o1 = results[2 * d + 1]["eemb_out"]
        eemb[d, :, 0:WLSE] = o0
        eemb[d, perm1, WLSE:HS] = o1

        row_of = _grid_row_table()
        g0 = results[2 * d]["rs_out"]
        g1 = results[2 * d + 1]["rs_out"]
        # canonical representative (mn, mx); mn<=20 lives on the even core,
        # mn>=21 on the odd core at slots (mn-21, mx-21)
        for i in range(NE):
            for j in range(NE):
                mn, mx = (i, j) if i <= j else (j, i)
                if mn < IL:
                    rs_grid[d, i, j] = g0[row_of[mn, mx]]
                else:
                    rs_grid[d, i, j] = g1[row_of[mn - IL, mx - IL]]

    hts_np = np.asarray(hts, dtype=np.int64)
    h_idx = hts_np[:, :, 0]
    t_idx = hts_np[:, :, 1]
    hss = np.empty([B, NE * NE, HS], np.float32)
    rss = np.empty([B, NE * NE, HS], np.float32)
    tss = np.empty([B, NE * NE, HS], np.float32)
    for d in range(B):
        hss[d] = eemb[d][h_idx[d]]
        tss[d] = eemb[d][t_idx[d]]
        rss[d] = rs_grid[d][h_idx[d], t_idx[d]]
    shape = (B, NE, NE, HS)
    return hss.reshape(shape), rss.reshape(shape), tss.reshape(shape)


def kernel(sequence_output, attention, entity_starts, hts):
    if "nc" not in _prog_cache:
        _prog_cache["nc"] = _build_program()
    nc = _prog_cache["nc"]

    in_maps = _host_inputs(sequence_output, attention, entity_starts)
    res = run_bass_kernel_spmd(nc, in_maps, list(range(N_CORES))).results
    return _assemble(res, entity_starts, hts)


if __name__ == "__main__":
    # smoke test with random data
    rng = np.random.default_rng(0)
    seq = rng.standard_normal((B, C, HS), dtype=np.float32)
    att = rng.random((B, H, C, C), dtype=np.float32)
    starts = rng.integers(0, 1020, (B, NE, M))
    hts = rng.integers(0, NE, (B, NE * NE, 2))
    outs = kernel(seq, att, starts, hts)
    print([o.shape for o in outs])



# revision 3
# speedup vs baseline: 1.3161x; 1.3161x over previous
"""Trainium2 Bass kernel for DocREModel_KD head (ragged_sequence).

Problem shape (hardcoded, per spec):
  sequence_output [4, 1024, 768] f32
  attention       [4, 12, 1024, 1024] f32
  entity_starts   [4, 42, 4] int
  hts             [4, 1764, 2] int
Outputs: (hss, rss, tss) each [4, 42, 42, 768] f32.

Strategy v2 (8 cores, SPMD single program, c-split + host reduce):
  - 2 cores per document, split by the attention column dim c (512 each).
    Each core gathers only its c-half of the mention attention rows (staged
    host-side as bf16 [pos, h, c-half], h-contiguous so one mention is one
    24KB/2 contiguous row), computes the full canonical pair grid G over its
    c-half (relu is elementwise-in-c after the h-sum, so each half is
    complete), and produces UNNORMALIZED partial rs plus a partial
    normalizer via a ones-column. The host adds the two halves and
    normalizes (the unshard step).
  - Canonical pair packing: 7 i-blocks of height 6, block b covers
    j in [6b, 42): U = 6*(42+36+30+24+18+12+6) = 1008 = 8 taus of 126.
  - EA (mention-mean of attention, c-partitioned) via per-(tile,head,chunk)
    PE matmuls against a tiny [84, 21] 0.25-selection matrix (mean +
    transpose in one step, bf16 weights -> fast LDWEIGHTS); the ACT drain
    un-interleaves h-major PSUM bands to the h-minor layout the DVE
    pair-products need for 2x mode.
  - Pair products on DVE (bf16 2x) + 12->6 first reduction level on DVE;
    lower tree levels + final add on GpSimd; relu on ScalarE.
  - rs = relu(G) @ [seq | ones] accumulated over the 4 c-chunks in PSUM,
    taus grouped 3/3/2 to fit banks; drains to bf16 and DMA out.
  - e_emb logsumexp d-split across the core pair (exact fp32), as before.
  - hss/tss and the hts->grid mapping assembled host-side.
"""

import numpy as np
from contextlib import ExitStack

import concourse.bass as bass
import concourse.bacc as bacc
import concourse.mybir as mybir
import concourse.tile as tile
from concourse.bass_utils import run_bass_kernel_spmd

# ---- problem constants ----
B, H, C, HS, NE, M = 4, 12, 1024, 768, 42, 4
OFFSET = 1
CH = C // 2          # 512: c-half per core
NCH = CH // 128      # 4 c-chunks per core
BH = 6               # i-block height
NB = NE // BH        # 7 blocks
BLKW = [NE - BH * b for b in range(NB)]            # 42,36,30,24,18,12,6
BLKOFF = [BH * sum(BLKW[:b]) for b in range(NB)]   # packed row offsets
U = BH * sum(BLKW)   # 1008 packed canonical pair rows
TAU = 126
NTAU = U // TAU      # 8
GT = 84              # mentions per gather tile (21 entities x 4)
HH = H // 2          # 6 heads per gather half
WLSE = HS // 2       # 384: e_emb d-split width per core
N_CORES = 8

F32 = mybir.dt.float32
BF16 = mybir.dt.bfloat16
I32 = mybir.dt.int32
NP_BF16 = mybir.dt.np(BF16)

# tree lower levels on gpsimd (fall back to vector if slow/unsupported)
TREE_GPSIMD = True

_prog_cache = {}


def _build_program():
    nc = bacc.Bacc(None)

    # att halves: [pos, (h, c)] rows; h-half hh covers heads 6*hh..6*hh+5
    att0 = nc.dram_tensor("att0", [C, HH * CH], BF16, kind="ExternalInput")
    att1 = nc.dram_tensor("att1", [C, HH * CH], BF16, kind="ExternalInput")
    seq_b = nc.dram_tensor("seq_b", [CH, HS], BF16, kind="ExternalInput")
    seq_lse = nc.dram_tensor("seq_lse", [C, WLSE], F32, kind="ExternalInput")
    sel_d = nc.dram_tensor("sel", [GT, NE // 2], BF16, kind="ExternalInput")
    idx_g_d = nc.dram_tensor("idx_g", [GT, 2], I32, kind="ExternalInput")
    idx_lse_d = nc.dram_tensor("idx_lse", [NE, M], I32, kind="ExternalInput")

    rs_out = nc.dram_tensor("rs_out", [U, HS + 1], BF16, kind="ExternalOutput")
    eemb_out = nc.dram_tensor("eemb_out", [NE, WLSE], F32, kind="ExternalOutput")

    atts = [att0, att1]

    with tile.TileContext(nc) as tc, ExitStack() as ctx:
        const_p = ctx.enter_context(tc.tile_pool(name="const", bufs=1))
        raw_p = ctx.enter_context(tc.tile_pool(name="raw", bufs=1))
        ea_p = ctx.enter_context(tc.tile_pool(name="ea", bufs=2))
        pr_p = ctx.enter_context(tc.tile_pool(name="pr", bufs=2))
        t6_p = ctx.enter_context(tc.tile_pool(name="t6", bufs=2))
        x2_p = ctx.enter_context(tc.tile_pool(name="x2", bufs=2))
        gs_p = ctx.enter_context(tc.tile_pool(name="gs", bufs=2))
        g_p = ctx.enter_context(tc.tile_pool(name="g", bufs=4))
        seqb_p = ctx.enter_context(tc.tile_pool(name="seqb", bufs=1))
        lse_p = ctx.enter_context(tc.tile_pool(name="lse", bufs=1))
        st_p = ctx.enter_context(tc.tile_pool(name="st", bufs=3))

        ea_ps = ctx.enter_context(tc.tile_pool(name="eaps", bufs=2, space="PSUM"))
        rsA_ps = ctx.enter_context(tc.tile_pool(name="rsA", bufs=3, space="PSUM"))
        rsB_ps = ctx.enter_context(tc.tile_pool(name="rsB", bufs=3, space="PSUM"))

        # --- constants / indices to SBUF ---
        ig_sb = const_p.tile([GT, 2], I32, name="ig_sb")
        nc.sync.dma_start(out=ig_sb[:], in_=idx_g_d[:])
        il_sb = const_p.tile([NE, M], I32, name="il_sb")
        nc.sync.dma_start(out=il_sb[:], in_=idx_lse_d[:])
        sel_sb = const_p.tile([GT, NE // 2], BF16, name="sel_sb")
        nc.sync.dma_start(out=sel_sb[:], in_=sel_d[:])

        # --- indirect gathers: attention mention rows (c-half, bf16) ---
        # raw[t][hh]: [84 mentions, 6 heads * 512 c]
        raws = [[None, None], [None, None]]
        for t in range(2):
            for hh in range(2):
                rt = raw_p.tile([GT, HH * CH], BF16, name=f"raw{t}{hh}")
                nc.gpsimd.indirect_dma_start(
                    out=rt[:],
                    out_offset=None,
                    in_=atts[hh][:],
                    in_offset=bass.IndirectOffsetOnAxis(ap=ig_sb[:, t : t + 1], axis=0),
                )
                raws[t][hh] = rt

        # --- e_emb logsumexp pipeline (d-split half, exact fp32) ---
        sg = []
        for r in range(M):
            g = lse_p.tile([NE, WLSE], F32, name=f"sg{r}")
            nc.gpsimd.indirect_dma_start(
                out=g[:],
                out_offset=None,
                in_=seq_lse[:],
                in_offset=bass.IndirectOffsetOnAxis(ap=il_sb[:, r : r + 1], axis=0),
            )
            sg.append(g)
        ex = []
        for r in range(M):
            e = lse_p.tile([NE, WLSE], F32, name=f"ex{r}")
            nc.scalar.activation(out=e[:], in_=sg[r][:], func=mybir.ActivationFunctionType.Exp)
            ex.append(e)
        s01 = lse_p.tile([NE, WLSE], F32, name="s01")
        s23 = lse_p.tile([NE, WLSE], F32, name="s23")
        nc.vector.tensor_add(out=s01[:], in0=ex[0][:], in1=ex[1][:])
        nc.vector.tensor_add(out=s23[:], in0=ex[2][:], in1=ex[3][:])
        nc.vector.tensor_add(out=s01[:], in0=s01[:], in1=s23[:])
        lse_res = lse_p.tile([NE, WLSE], F32, name="lse_res")
        nc.scalar.activation(out=lse_res[:], in_=s01[:], func=mybir.ActivationFunctionType.Ln)
        nc.scalar.dma_start(out=eemb_out[:], in_=lse_res[:])

        # --- sequence chunks (already bf16) + ones column ---
        seqb = []
        for k in range(NCH):
            sb = seqb_p.tile([128, HS + 1], BF16, name=f"sb{k}")
            nc.sync.dma_start(out=sb[:, 0:HS], in_=seq_b[k * 128 : (k + 1) * 128, :])
            nc.vector.memset(sb[:, HS : HS + 1], 1.0)
            seqb.append(sb)

        # --- per c-chunk: EA transpose-mean, pair products, h-reduction ---
        gs = []
        for k in range(NCH):
            # EA: PSUM [128 c, (h-major) 12*42], 24 tiny matmuls
            ps = ea_ps.tile([128, H * NE], F32, name="eaps")
            for t in range(2):
                for h in range(H):
                    hh, hl = h // HH, h % HH
                    nc.tensor.matmul(
                        out=ps[:, h * NE + t * 21 : h * NE + t * 21 + 21],
                        lhsT=raws[t][hh][:, hl * CH + k * 128 : hl * CH + (k + 1) * 128],
                        rhs=sel_sb[:],
                        start=True,
                        stop=True,
                    )
            # drain + un-interleave to h-minor bf16 [128, (n, h)]
            ea = ea_p.tile([128, NE * H], BF16, name=f"ea{k}")
            nc.scalar.activation(
                out=ea[:].rearrange("p (n h) -> p n h", h=H),
                in_=ps[:].rearrange("p (h n) -> p n h", n=NE),
                func=mybir.ActivationFunctionType.Copy,
            )

            # pair products (DVE bf16 2x): pr[p, (u, h)]
            ea3 = ea[:].rearrange("p (n h) -> p n h", h=H)
            pr = pr_p.tile([128, U * H], BF16, name="pr")
            for b in range(NB):
                w = BLKW[b]
                jf = BH * b
                in0 = ea3[:, jf : jf + BH, :].unsqueeze(2).to_broadcast([128, BH, w, H])
                in1 = ea3[:, jf:NE, :].unsqueeze(1).to_broadcast([128, BH, w, H])
                sec = pr[:, BLKOFF[b] * H : (BLKOFF[b] + BH * w) * H]
                pr4 = sec.rearrange("p (i j h) -> p i j h", j=w, h=H)
                nc.vector.tensor_tensor(out=pr4, in0=in0, in1=in1, op=mybir.AluOpType.mult)

            # L1: 12 -> 6 on DVE (2x, aligned)
            pru = pr[:].rearrange("p (u h) -> p u h", h=H)
            t6 = t6_p.tile([128, U * 6], BF16, name="t6")
            t6v = t6[:].rearrange("p (u s) -> p u s", s=6)
            nc.vector.tensor_tensor(out=t6v, in0=pru[:, :, 0:6], in1=pru[:, :, 6:12], op=mybir.AluOpType.add)

            # L2: 6 -> 2 (two adds, 4B-aligned) ; L3: 2 -> 1
            eng = nc.gpsimd if TREE_GPSIMD else nc.vector
            x2 = x2_p.tile([128, U * 2], BF16, name="x2")
            x2v = x2[:].rearrange("p (u s) -> p u s", s=2)
            eng.tensor_tensor(out=x2v, in0=t6v[:, :, 0:2], in1=t6v[:, :, 2:4], op=mybir.AluOpType.add)
            eng.tensor_tensor(out=x2v, in0=x2v, in1=t6v[:, :, 4:6], op=mybir.AluOpType.add)
            gsum = gs_p.tile([128, U], BF16, name="gsum")
            a = x2v[:, :, 0:1].squeeze(2)
            bb = x2v[:, :, 1:2].squeeze(2)
            eng.tensor_tensor(out=gsum[:], in0=a, in1=bb, op=mybir.AluOpType.add)

            # relu on ACT
            g_t = g_p.tile([128, U], BF16, name=f"g{k}")
            nc.scalar.activation(out=g_t[:], in_=gsum[:], func=mybir.ActivationFunctionType.Relu)
            gs.append(g_t)

        # --- rs matmuls: taus grouped to fit PSUM banks; accumulate over k ---
        groups = [(0, 1, 2), (3, 4, 5), (6, 7)]
        for grp in groups:
            psAs, psBs = {}, {}
            for tau in grp:
                psAs[tau] = rsA_ps.tile([TAU, 512], F32, name="psA")
                psBs[tau] = rsB_ps.tile([TAU, HS + 1 - 512], F32, name="psB")
            for k in range(NCH):
                for tau in grp:
                    lo = tau * TAU
                    nc.tensor.matmul(
                        out=psAs[tau][:],
                        lhsT=gs[k][:, lo : lo + TAU],
                        rhs=seqb[k][:, 0:512],
                        start=(k == 0),
                        stop=(k == NCH - 1),
                    )
                    nc.tensor.matmul(
                        out=psBs[tau][:],
                        lhsT=gs[k][:, lo : lo + TAU],
                        rhs=seqb[k][:, 512 : HS + 1],
                        start=(k == 0),
                        stop=(k == NCH - 1),
                    )
            for tau in grp:
                lo = tau * TAU
                st = st_p.tile([TAU, HS + 1], BF16, name="st")
                nc.scalar.activation(
                    out=st[:, 0:512], in_=psAs[tau][:],
                    func=mybir.ActivationFunctionType.Copy,
                )
                nc.scalar.activation(
                    out=st[:, 512 : HS + 1], in_=psBs[tau][:],
                    func=mybir.ActivationFunctionType.Copy,
                )
                nc.sync.dma_start(out=rs_out[lo : lo + TAU, :], in_=st[:])

    nc.finalize()
    return nc


def _host_inputs(sequence_output, attention, entity_starts):
    """Build the 8 per-core input maps."""
    sel_np = np.zeros([GT, NE // 2], np.float32)
    sel_np[np.arange(GT), np.arange(GT) // M] = 0.25
    sel_np = sel_np.astype(NP_BF16)

    in_maps = []
    for d in range(B):
        starts_doc = np.asarray(entity_starts[d], dtype=np.int64)
        pos = (starts_doc + OFFSET).astype(np.int32)      # [42, 4], < 1024

        ig = np.zeros([GT, 2], np.int32)
        for t in range(2):
            ig[:, t] = pos[21 * t + np.arange(GT) // M, np.arange(GT) % M]

        att_bf = np.asarray(attention[d], dtype=np.float32).astype(NP_BF16)  # [12,1024,1024]
        att_t = att_bf.transpose(1, 0, 2)                 # [pos, h, c]
        seq_doc = np.asarray(sequence_output[d], dtype=np.float32)

        for ch in range(2):
            csl = slice(ch * CH, (ch + 1) * CH)
            att_half = np.ascontiguousarray(att_t[:, :, csl])   # [1024, 12, 512]
            att_half = att_half.reshape(C, H * CH)
            in_maps.append(
                {
                    "att0": np.ascontiguousarray(att_half[:, 0 : HH * CH]),
                    "att1": np.ascontiguousarray(att_half[:, HH * CH :]),
                    "seq_b": np.ascontiguousarray(seq_doc[csl, :]).astype(NP_BF16),
                    "seq_lse": np.ascontiguousarray(
                        seq_doc[:, ch * WLSE : (ch + 1) * WLSE]
                    ),
                    "sel": sel_np,
                    "idx_g": ig,
                    "idx_lse": pos,
                }
            )
    return in_maps


_row_table_cache = {}


def _grid_row_table():
    """[42, 42] -> packed canonical row (use at [min, max])."""
    if "t" not in _row_table_cache:
        row_of = np.full((NE, NE), -1, np.int64)
        for bb in range(NB):
            w = BLKW[bb]
            jf = BH * bb
            for il in range(BH):
                for j in range(jf, NE):
                    row_of[jf + il, j] = BLKOFF[bb] + il * w + (j - jf)
        _row_table_cache["t"] = row_of
    return _row_table_cache["t"]


def _assemble(results, hts):
    eemb = np.empty([B, NE, HS], np.float32)
    rs_rows = np.empty([B, U, HS], np.float32)
    row_of = _grid_row_table()
    for d in range(B):
        o0 = results[2 * d]["eemb_out"]
        o1 = results[2 * d + 1]["eemb_out"]
        eemb[d, :, 0:WLSE] = o0
        eemb[d, :, WLSE:HS] = o1

        p0 = np.asarray(results[2 * d]["rs_out"], dtype=np.float32)
        p1 = np.asarray(results[2 * d + 1]["rs_out"], dtype=np.float32)
        s = p0 + p1                                   # [1008, 769]
        rs_rows[d] = s[:, 0:HS] / (s[:, HS : HS + 1] + 1e-10)

    hts_np = np.asarray(hts, dtype=np.int64)
    h_idx = hts_np[:, :, 0]                            # [B, 1764]
    t_idx = hts_np[:, :, 1]
    mn = np.minimum(h_idx, t_idx)
    mx = np.maximum(h_idx, t_idx)
    shape = (B, NE, NE, HS)
    hss = np.empty([B, NE * NE, HS], np.float32)
    rss = np.empty([B, NE * NE, HS], np.float32)
    tss = np.empty([B, NE * NE, HS], np.float32)
    for d in range(B):
        hss[d] = eemb[d][h_idx[d]]
        tss[d] = eemb[d][t_idx[d]]
        rss[d] = rs_rows[d][row_of[mn[d], mx[d]]]
    return hss.reshape(shape), rss.reshape(shape), tss.reshape(shape)


def kernel(sequence_output, attention, entity_starts, hts):
    if "nc" not in _prog_cache:
        _prog_cache["nc"] = _build_program()
    nc = _prog_cache["nc"]

    in_maps = _host_inputs(sequence_output, attention, entity_starts)
    res = run_bass_kernel_spmd(nc, in_maps, list(range(N_CORES))).results
    return _assemble(res, hts)


if __name__ == "__main__":
    # smoke test with random data
    rng = np.random.default_rng(0)
    seq = rng.standard_normal((B, C, HS), dtype=np.float32)
    att = rng.random((B, H, C, C), dtype=np.float32)
    starts = rng.integers(0, 1020, (B, NE, M))
    hts_a = rng.integers(0, NE, (B, NE * NE, 2))
    outs = kernel(seq, att, starts, hts_a)
    print([o.shape for o in outs])


# revision 18
# speedup vs baseline: 1.5756x; 1.1972x over previous
"""Trainium2 Bass kernel for DocREModel_KD head (ragged_sequence).

Problem shape (hardcoded, per spec):
  sequence_output [4, 1024, 768] f32
  attention       [4, 12, 1024, 1024] f32
  entity_starts   [4, 42, 4] int
  hts             [4, 1764, 2] int
Outputs: (hss, rss, tss) each [4, 42, 42, 768] f32.

Strategy v2 (8 cores, SPMD single program, c-split + host reduce):
  - 2 cores per document, split by the attention column dim c (512 each).
    Each core gathers only its c-half of the mention attention rows (staged
    host-side as bf16 [pos, h, c-half], h-contiguous so one mention is one
    24KB/2 contiguous row), computes the full canonical pair grid G over its
    c-half (relu is elementwise-in-c after the h-sum, so each half is
    complete), and produces UNNORMALIZED partial rs plus a partial
    normalizer via a ones-column. The host adds the two halves and
    normalizes (the unshard step).
  - Canonical pair packing: 7 i-blocks of height 6, block b covers
    j in [6b, 42): U = 6*(42+36+30+24+18+12+6) = 1008 = 8 taus of 126.
  - EA (mention-mean of attention, c-partitioned) via per-(tile,head,chunk)
    PE matmuls against a tiny [84, 21] 0.25-selection matrix (mean +
    transpose in one step, bf16 weights -> fast LDWEIGHTS); the ACT drain
    un-interleaves h-major PSUM bands to the h-minor layout the DVE
    pair-products need for 2x mode.
  - Pair products on DVE (bf16 2x) + 12->6 first reduction level on DVE;
    lower tree levels + final add on GpSimd; relu on ScalarE.
  - rs = relu(G) @ [seq | ones] accumulated over the 4 c-chunks in PSUM,
    taus grouped 3/3/2 to fit banks; drains to bf16 and DMA out.
  - e_emb logsumexp d-split across the core pair (exact fp32), as before.
  - hss/tss and the hts->grid mapping assembled host-side.
"""

import numpy as np
from contextlib import ExitStack

import concourse.bass as bass
import concourse.bacc as bacc
import concourse.mybir as mybir
import concourse.tile as tile
from concourse.bass_utils import run_bass_kernel_spmd

# ---- problem constants ----
B, H, C, HS, NE, M = 4, 12, 1024, 768, 42, 4
OFFSET = 1
CH = C // 2          # 512: c-half per core
NCH = CH // 128      # 4 c-chunks per core
BH = 6               # i-block height
NB = NE // BH        # 7 blocks
BLKW = [NE - BH * b for b in range(NB)]            # 42,36,30,24,18,12,6
BLKOFF = [BH * sum(BLKW[:b]) for b in range(NB)]   # packed row offsets
U = BH * sum(BLKW)   # 1008 packed canonical pair rows
TAU = 126
NTAU = U // TAU      # 8
GT = 84              # mentions per gather tile (21 entities x 4)
HH = H // 2          # 6 heads per gather half
WLSE = HS // 2       # 384: e_emb d-split width per core
N_CORES = 8

F32 = mybir.dt.float32
BF16 = mybir.dt.bfloat16
I32 = mybir.dt.int32
NP_BF16 = mybir.dt.np(BF16)

# tree lower levels on gpsimd (measured: gpsimd tensor_tensor is ~0.4 elem/cyc
# and contends with DVE on SBUF ports — keep False)
TREE_GPSIMD = False
UTAU = 128           # padded tau width (G padded to 1024 rows for FWL)
NTAU_P = 8

_prog_cache = {}


def _build_program():
    nc = bacc.Bacc(None)

    # att halves: [pos, (h, c)] rows; h-half hh covers heads 6*hh..6*hh+5
    att0 = nc.dram_tensor("att0", [C, HH * CH], BF16, kind="ExternalInput")
    att1 = nc.dram_tensor("att1", [C, HH * CH], BF16, kind="ExternalInput")
    seq_b = nc.dram_tensor("seq_b", [CH, HS], BF16, kind="ExternalInput")
    seq_lse = nc.dram_tensor("seq_lse", [C, WLSE], F32, kind="ExternalInput")
    sel_d = nc.dram_tensor("sel", [GT, NE // 2], BF16, kind="ExternalInput")
    idx_g_d = nc.dram_tensor("idx_g", [GT, 2], I32, kind="ExternalInput")
    idx_lse_d = nc.dram_tensor("idx_lse", [NE, M], I32, kind="ExternalInput")

    rs_out = nc.dram_tensor("rs_out", [NTAU_P * UTAU, HS + 1], BF16, kind="ExternalOutput")
    eemb_out = nc.dram_tensor("eemb_out", [NE, WLSE], F32, kind="ExternalOutput")

    atts = [att0, att1]

    with tile.TileContext(nc) as tc, ExitStack() as ctx:
        const_p = ctx.enter_context(tc.tile_pool(name="const", bufs=1))
        raw_p = ctx.enter_context(tc.tile_pool(name="raw", bufs=1))
        ea_p = ctx.enter_context(tc.tile_pool(name="ea", bufs=2))
        pr_p = ctx.enter_context(tc.tile_pool(name="pr", bufs=2))
        t6_p = ctx.enter_context(tc.tile_pool(name="t6", bufs=2))
        x2_p = ctx.enter_context(tc.tile_pool(name="x2", bufs=2))
        gs_p = ctx.enter_context(tc.tile_pool(name="gs", bufs=2))
        g_p = ctx.enter_context(tc.tile_pool(name="g", bufs=4))
        seqb_p = ctx.enter_context(tc.tile_pool(name="seqb", bufs=1))
        lse_p = ctx.enter_context(tc.tile_pool(name="lse", bufs=1))
        st_p = ctx.enter_context(tc.tile_pool(name="st", bufs=3))

        ea_ps = ctx.enter_context(tc.tile_pool(name="eaps", bufs=2, space="PSUM"))
        rsA_ps = ctx.enter_context(tc.tile_pool(name="rsA", bufs=3, space="PSUM"))
        rsB_ps = ctx.enter_context(tc.tile_pool(name="rsB", bufs=3, space="PSUM"))

        # --- constants / indices to SBUF ---
        ig_sb = const_p.tile([GT, 2], I32, name="ig_sb")
        nc.sync.dma_start(out=ig_sb[:], in_=idx_g_d[:])
        il_sb = const_p.tile([NE, M], I32, name="il_sb")
        nc.sync.dma_start(out=il_sb[:], in_=idx_lse_d[:])
        sel_sb = const_p.tile([GT, NE // 2], BF16, name="sel_sb")
        nc.sync.dma_start(out=sel_sb[:], in_=sel_d[:])

        # --- indirect gathers: attention mention rows (c-half, bf16) ---
        # raws[t][hh]: [84 mentions, 6 heads * 512 c]
        raws = [[None, None], [None, None]]
        for t in range(2):
            for hh in range(2):
                rt = raw_p.tile([GT, HH * CH], BF16, name=f"raw{t}{hh}")
                nc.gpsimd.indirect_dma_start(
                    out=rt[:],
                    out_offset=None,
                    in_=atts[hh][:],
                    in_offset=bass.IndirectOffsetOnAxis(ap=ig_sb[:, t : t + 1], axis=0),
                )
                raws[t][hh] = rt

        # --- e_emb logsumexp pipeline (d-split half, exact fp32) ---
        sg = []
        for r in range(M):
            g = lse_p.tile([NE, WLSE], F32, name=f"sg{r}")
            nc.gpsimd.indirect_dma_start(
                out=g[:],
                out_offset=None,
                in_=seq_lse[:],
                in_offset=bass.IndirectOffsetOnAxis(ap=il_sb[:, r : r + 1], axis=0),
            )
            sg.append(g)
        ex = []
        for r in range(M):
            e = lse_p.tile([NE, WLSE], F32, name=f"ex{r}")
            nc.scalar.activation(out=e[:], in_=sg[r][:], func=mybir.ActivationFunctionType.Exp)
            ex.append(e)
        s01 = lse_p.tile([NE, WLSE], F32, name="s01")
        s23 = lse_p.tile([NE, WLSE], F32, name="s23")
        nc.vector.tensor_add(out=s01[:], in0=ex[0][:], in1=ex[1][:])
        nc.vector.tensor_add(out=s23[:], in0=ex[2][:], in1=ex[3][:])
        nc.vector.tensor_add(out=s01[:], in0=s01[:], in1=s23[:])
        lse_res = lse_p.tile([NE, WLSE], F32, name="lse_res")
        nc.scalar.activation(out=lse_res[:], in_=s01[:], func=mybir.ActivationFunctionType.Ln)
        nc.scalar.dma_start(out=eemb_out[:], in_=lse_res[:])

        # --- sequence chunks (already bf16) + ones column ---
        seqb = []
        for k in range(NCH):
            sb = seqb_p.tile([128, HS + 1], BF16, name=f"sb{k}")
            nc.sync.dma_start(out=sb[:, 0:HS], in_=seq_b[k * 128 : (k + 1) * 128, :])
            nc.vector.memset(sb[:, HS : HS + 1], 1.0)
            seqb.append(sb)

        # --- per c-chunk: EA transpose-mean, pair products, h-reduction ---
        gs = []
        for k in range(NCH):
            # EA: PSUM [128 c, (h-major) 12*42], 24 tiny matmuls
            ps = ea_ps.tile([128, H * NE], F32, name="eaps")
            for t in range(2):
                for h in range(H):
                    hh, hl = h // HH, h % HH
                    nc.tensor.matmul(
                        out=ps[:, h * NE + t * 21 : h * NE + t * 21 + 21],
                        lhsT=raws[t][hh][:, hl * CH + k * 128 : hl * CH + (k + 1) * 128],
                        rhs=sel_sb[:],
                        start=True,
                        stop=True,
                    )
            # drain + un-interleave to h-minor bf16 [128, (n, h)]
            ea = ea_p.tile([128, NE * H], BF16, name=f"ea{k}")
            nc.scalar.activation(
                out=ea[:].rearrange("p (n h) -> p n h", h=H),
                in_=ps[:].rearrange("p (h n) -> p n h", n=NE),
                func=mybir.ActivationFunctionType.Copy,
            )

            # pair products (DVE bf16 2x): pr[p, (u, h)]
            ea3 = ea[:].rearrange("p (n h) -> p n h", h=H)
            pr = pr_p.tile([128, U * H], BF16, name="pr")
            for b in range(NB):
                w = BLKW[b]
                jf = BH * b
                in0 = ea3[:, jf : jf + BH, :].unsqueeze(2).to_broadcast([128, BH, w, H])
                in1 = ea3[:, jf:NE, :].unsqueeze(1).to_broadcast([128, BH, w, H])
                sec = pr[:, BLKOFF[b] * H : (BLKOFF[b] + BH * w) * H]
                pr4 = sec.rearrange("p (i j h) -> p i j h", j=w, h=H)
                nc.vector.tensor_tensor(out=pr4, in0=in0, in1=in1, op=mybir.AluOpType.mult)

            # L1: 12 -> 6 on DVE (2x, aligned)
            pru = pr[:].rearrange("p (u h) -> p u h", h=H)
            t6 = t6_p.tile([128, U * 6], BF16, name="t6")
            t6v = t6[:].rearrange("p (u s) -> p u s", s=6)
            nc.vector.tensor_tensor(out=t6v, in0=pru[:, :, 0:6], in1=pru[:, :, 6:12], op=mybir.AluOpType.add)

            # L2: 6 -> 2 (two adds, 4B-aligned) ; L3: 2 -> 1
            eng = nc.gpsimd if TREE_GPSIMD else nc.vector
            x2 = x2_p.tile([128, U * 2], BF16, name="x2")
            x2v = x2[:].rearrange("p (u s) -> p u s", s=2)
            eng.tensor_tensor(out=x2v, in0=t6v[:, :, 0:2], in1=t6v[:, :, 2:4], op=mybir.AluOpType.add)
            eng.tensor_tensor(out=x2v, in0=x2v, in1=t6v[:, :, 4:6], op=mybir.AluOpType.add)
            gsum = gs_p.tile([128, U], BF16, name="gsum")
            a = x2v[:, :, 0:1].squeeze(2)
            bb = x2v[:, :, 1:2].squeeze(2)
            eng.tensor_tensor(out=gsum[:], in0=a, in1=bb, op=mybir.AluOpType.add)

            # relu on ACT; pad to 1024 rows so rs taus are 128 wide (FWL)
            g_t = g_p.tile([128, NTAU_P * UTAU], BF16, name=f"g{k}")
            nc.gpsimd.memset(g_t[:, U:], 0.0)
            nc.scalar.activation(out=g_t[:, 0:U], in_=gsum[:], func=mybir.ActivationFunctionType.Relu)
            gs.append(g_t)

        # --- rs matmuls: taus grouped to fit PSUM banks; accumulate over k ---
        groups = [(0, 1, 2), (3, 4, 5), (6, 7)]
        for grp in groups:
            psAs, psBs = {}, {}
            for tau in grp:
                psAs[tau] = rsA_ps.tile([UTAU, 512], F32, name="psA")
                psBs[tau] = rsB_ps.tile([UTAU, HS + 1 - 512], F32, name="psB")
            for k in range(NCH):
                for tau in grp:
                    lo = tau * UTAU
                    nc.tensor.matmul(
                        out=psAs[tau][:],
                        lhsT=gs[k][:, lo : lo + UTAU],
                        rhs=seqb[k][:, 0:512],
                        start=(k == 0),
                        stop=(k == NCH - 1),
                    )
                    nc.tensor.matmul(
                        out=psBs[tau][:],
                        lhsT=gs[k][:, lo : lo + UTAU],
                        rhs=seqb[k][:, 512 : HS + 1],
                        start=(k == 0),
                        stop=(k == NCH - 1),
                    )
            for tau in grp:
                lo = tau * UTAU
                st = st_p.tile([UTAU, HS + 1], BF16, name="st")
                nc.scalar.activation(
                    out=st[:, 0:512], in_=psAs[tau][:],
                    func=mybir.ActivationFunctionType.Copy,
                )
                nc.scalar.activation(
                    out=st[:, 512 : HS + 1], in_=psBs[tau][:],
                    func=mybir.ActivationFunctionType.Copy,
                )
                nc.sync.dma_start(out=rs_out[lo : lo + UTAU, :], in_=st[:])

    nc.finalize()
    return nc


def _host_inputs(sequence_output, attention, entity_starts):
    """Build the 8 per-core input maps."""
    sel_np = np.zeros([GT, NE // 2], np.float32)
    sel_np[np.arange(GT), np.arange(GT) // M] = 0.25
    sel_np = sel_np.astype(NP_BF16)

    in_maps = []
    for d in range(B):
        starts_doc = np.asarray(entity_starts[d], dtype=np.int64)
        pos = (starts_doc + OFFSET).astype(np.int32)      # [42, 4], < 1024

        ig = np.zeros([GT, 2], np.int32)
        for t in range(2):
            ig[:, t] = pos[21 * t + np.arange(GT) // M, np.arange(GT) % M]

        att_bf = np.asarray(attention[d], dtype=np.float32).astype(NP_BF16)  # [12,1024,1024]
        att_t = att_bf.transpose(1, 0, 2)                 # [pos, h, c]
        seq_doc = np.asarray(sequence_output[d], dtype=np.float32)

        for ch in range(2):
            csl = slice(ch * CH, (ch + 1) * CH)
            att_half = np.ascontiguousarray(att_t[:, :, csl])   # [1024, 12, 512]
            att_half = att_half.reshape(C, H * CH)
            in_maps.append(
                {
                    "att0": np.ascontiguousarray(att_half[:, 0 : HH * CH]),
                    "att1": np.ascontiguousarray(att_half[:, HH * CH :]),
                    "seq_b": np.ascontiguousarray(seq_doc[csl, :]).astype(NP_BF16),
                    "seq_lse": np.ascontiguousarray(
                        seq_doc[:, ch * WLSE : (ch + 1) * WLSE]
                    ),
                    "sel": sel_np,
                    "idx_g": ig,
                    "idx_lse": pos,
                }
            )
    return in_maps


_row_table_cache = {}


def _grid_row_table():
    """[42, 42] -> packed canonical row (use at [min, max])."""
    if "t" not in _row_table_cache:
        row_of = np.full((NE, NE), -1, np.int64)
        for bb in range(NB):
            w = BLKW[bb]
            jf = BH * bb
            for il in range(BH):
                for j in range(jf, NE):
                    row_of[jf + il, j] = BLKOFF[bb] + il * w + (j - jf)
        _row_table_cache["t"] = row_of
    return _row_table_cache["t"]


def _assemble(results, hts):
    eemb = np.empty([B, NE, HS], np.float32)
    rs_rows = np.empty([B, U, HS], np.float32)
    row_of = _grid_row_table()
    for d in range(B):
        o0 = results[2 * d]["eemb_out"]
        o1 = results[2 * d + 1]["eemb_out"]
        eemb[d, :, 0:WLSE] = o0
        eemb[d, :, WLSE:HS] = o1

        p0 = np.asarray(results[2 * d]["rs_out"], dtype=np.float32)[:U]
        p1 = np.asarray(results[2 * d + 1]["rs_out"], dtype=np.float32)[:U]
        s = p0 + p1                                   # [1008, 769]
        rs_rows[d] = s[:, 0:HS] / (s[:, HS : HS + 1] + 1e-10)

    hts_np = np.asarray(hts, dtype=np.int64)
    h_idx = hts_np[:, :, 0]                            # [B, 1764]
    t_idx = hts_np[:, :, 1]
    mn = np.minimum(h_idx, t_idx)
    mx = np.maximum(h_idx, t_idx)
    shape = (B, NE, NE, HS)
    hss = np.empty([B, NE * NE, HS], np.float32)
    rss = np.empty([B, NE * NE, HS], np.float32)
    tss = np.empty([B, NE * NE, HS], np.float32)
    for d in range(B):
        hss[d] = eemb[d][h_idx[d]]
        tss[d] = eemb[d][t_idx[d]]
        rss[d] = rs_rows[d][row_of[mn[d], mx[d]]]
    return hss.reshape(shape), rss.reshape(shape), tss.reshape(shape)


def kernel(sequence_output, attention, entity_starts, hts):
    if "nc" not in _prog_cache:
        _prog_cache["nc"] = _build_program()
    nc = _prog_cache["nc"]

    in_maps = _host_inputs(sequence_output, attention, entity_starts)
    res = run_bass_kernel_spmd(nc, in_maps, list(range(N_CORES))).results
    return _assemble(res, hts)


if __name__ == "__main__":
    # smoke test with random data
    rng = np.random.default_rng(0)
    seq = rng.standard_normal((B, C, HS), dtype=np.float32)
    att = rng.random((B, H, C, C), dtype=np.float32)
    starts = rng.integers(0, 1020, (B, NE, M))
    hts_a = rng.integers(0, NE, (B, NE * NE, 2))
    outs = kernel(seq, att, starts, hts_a)
    print([o.shape for o in outs])


# revision 24
# speedup vs baseline: 1.5897x; 1.0089x over previous
"""Trainium2 Bass kernel for DocREModel_KD head (ragged_sequence).

Problem shape (hardcoded, per spec):
  sequence_output [4, 1024, 768] f32
  attention       [4, 12, 1024, 1024] f32
  entity_starts   [4, 42, 4] int
  hts             [4, 1764, 2] int
Outputs: (hss, rss, tss) each [4, 42, 42, 768] f32.

Strategy v2 (8 cores, SPMD single program, c-split + host reduce):
  - 2 cores per document, split by the attention column dim c (512 each).
    Each core gathers only its c-half of the mention attention rows (staged
    host-side as bf16 [pos, h, c-half], h-contiguous so one mention is one
    24KB/2 contiguous row), computes the full canonical pair grid G over its
    c-half (relu is elementwise-in-c after the h-sum, so each half is
    complete), and produces UNNORMALIZED partial rs plus a partial
    normalizer via a ones-column. The host adds the two halves and
    normalizes (the unshard step).
  - Canonical pair packing: 7 i-blocks of height 6, block b covers
    j in [6b, 42): U = 6*(42+36+30+24+18+12+6) = 1008 = 8 taus of 126.
  - EA (mention-mean of attention, c-partitioned) via per-(tile,head,chunk)
    PE matmuls against a tiny [84, 21] 0.25-selection matrix (mean +
    transpose in one step, bf16 weights -> fast LDWEIGHTS); the ACT drain
    un-interleaves h-major PSUM bands to the h-minor layout the DVE
    pair-products need for 2x mode.
  - Pair products on DVE (bf16 2x) + 12->6 first reduction level on DVE;
    lower tree levels + final add on GpSimd; relu on ScalarE.
  - rs = relu(G) @ [seq | ones] accumulated over the 4 c-chunks in PSUM,
    taus grouped 3/3/2 to fit banks; drains to bf16 and DMA out.
  - e_emb logsumexp d-split across the core pair (exact fp32), as before.
  - hss/tss and the hts->grid mapping assembled host-side.
"""

import numpy as np
from contextlib import ExitStack

import concourse.bass as bass
import concourse.bacc as bacc
import concourse.mybir as mybir
import concourse.tile as tile
from concourse.bass_utils import run_bass_kernel_spmd

# ---- problem constants ----
B, H, C, HS, NE, M = 4, 12, 1024, 768, 42, 4
OFFSET = 1
CH = C // 2          # 512: c-half per core
NCH = CH // 128      # 4 c-chunks per core
BH = 6               # i-block height
NB = NE // BH        # 7 blocks
BLKW = [NE - BH * b for b in range(NB)]            # 42,36,30,24,18,12,6
BLKOFF = [BH * sum(BLKW[:b]) for b in range(NB)]   # packed row offsets
U = BH * sum(BLKW)   # 1008 packed canonical pair rows
TAU = 126
NTAU = U // TAU      # 8
GT = 84              # mentions per gather tile (21 entities x 4)
HH = H // 2          # 6 heads per gather half
WLSE = HS // 2       # 384: e_emb d-split width per core
N_CORES = 8

F32 = mybir.dt.float32
BF16 = mybir.dt.bfloat16
I32 = mybir.dt.int32
NP_BF16 = mybir.dt.np(BF16)

# tree lower levels on gpsimd (measured: gpsimd tensor_tensor is ~0.4 elem/cyc
# and contends with DVE on SBUF ports — keep False)
TREE_GPSIMD = False
UTAU = 128           # padded tau width (G padded to 1024 rows for FWL)
NTAU_P = 8

_prog_cache = {}


def _build_program():
    nc = bacc.Bacc(None)

    # att halves: [pos, (h, c)] rows; h-half hh covers heads 6*hh..6*hh+5
    att = nc.dram_tensor("att", [C, H * CH], BF16, kind="ExternalInput")
    seq_b = nc.dram_tensor("seq_b", [CH, HS], BF16, kind="ExternalInput")
    seq_lse = nc.dram_tensor("seq_lse", [C, WLSE], F32, kind="ExternalInput")
    sel_d = nc.dram_tensor("sel", [GT, NE // 2], BF16, kind="ExternalInput")
    idx_g_d = nc.dram_tensor("idx_g", [GT, 2], I32, kind="ExternalInput")
    idx_lse_d = nc.dram_tensor("idx_lse", [NE, M], I32, kind="ExternalInput")

    # per-chunk unnormalized rs partials (+ ones-column), host-reduced
    rs_out = nc.dram_tensor(
        "rs_out", [NCH * NTAU_P * UTAU, HS + 1], BF16, kind="ExternalOutput"
    )
    eemb_out = nc.dram_tensor("eemb_out", [NE, WLSE], F32, kind="ExternalOutput")

    with tile.TileContext(nc) as tc, ExitStack() as ctx:
        const_p = ctx.enter_context(tc.tile_pool(name="const", bufs=1))
        raw_p = ctx.enter_context(tc.tile_pool(name="raw", bufs=1))
        ea_p = ctx.enter_context(tc.tile_pool(name="ea", bufs=2))
        pr_p = ctx.enter_context(tc.tile_pool(name="pr", bufs=2))
        t6_p = ctx.enter_context(tc.tile_pool(name="t6", bufs=2))
        x2_p = ctx.enter_context(tc.tile_pool(name="x2", bufs=2))
        gs_p = ctx.enter_context(tc.tile_pool(name="gs", bufs=2))
        g_p = ctx.enter_context(tc.tile_pool(name="g", bufs=4))
        seqb_p = ctx.enter_context(tc.tile_pool(name="seqb", bufs=1))
        lse_p = ctx.enter_context(tc.tile_pool(name="lse", bufs=1))
        st_p = ctx.enter_context(tc.tile_pool(name="st", bufs=3))

        ea_ps = ctx.enter_context(tc.tile_pool(name="eaps", bufs=2, space="PSUM"))
        rsA_ps = ctx.enter_context(tc.tile_pool(name="rsA", bufs=3, space="PSUM"))
        rsB_ps = ctx.enter_context(tc.tile_pool(name="rsB", bufs=3, space="PSUM"))

        # --- constants / indices to SBUF ---
        ig_sb = const_p.tile([GT, 2], I32, name="ig_sb")
        nc.sync.dma_start(out=ig_sb[:], in_=idx_g_d[:])
        il_sb = const_p.tile([NE, M], I32, name="il_sb")
        nc.sync.dma_start(out=il_sb[:], in_=idx_lse_d[:])
        sel_sb = const_p.tile([GT, NE // 2], BF16, name="sel_sb")
        nc.sync.dma_start(out=sel_sb[:], in_=sel_d[:])

        # --- indirect gathers: attention mention rows (c-half, bf16) ---
        # raws[t]: [84 mentions, 12 heads * 512 c] (12KB contiguous rows)
        raws = []
        for t in range(2):
            rt = raw_p.tile([GT, H * CH], BF16, name=f"raw{t}")
            nc.gpsimd.indirect_dma_start(
                out=rt[:],
                out_offset=None,
                in_=att[:],
                in_offset=bass.IndirectOffsetOnAxis(ap=ig_sb[:, t : t + 1], axis=0),
            )
            raws.append(rt)

        # --- e_emb logsumexp pipeline (d-split half, exact fp32) ---
        sg = []
        for r in range(M):
            g = lse_p.tile([NE, WLSE], F32, name=f"sg{r}")
            nc.gpsimd.indirect_dma_start(
                out=g[:],
                out_offset=None,
                in_=seq_lse[:],
                in_offset=bass.IndirectOffsetOnAxis(ap=il_sb[:, r : r + 1], axis=0),
            )
            sg.append(g)
        ex = []
        for r in range(M):
            e = lse_p.tile([NE, WLSE], F32, name=f"ex{r}")
            nc.scalar.activation(out=e[:], in_=sg[r][:], func=mybir.ActivationFunctionType.Exp)
            ex.append(e)
        s01 = lse_p.tile([NE, WLSE], F32, name="s01")
        s23 = lse_p.tile([NE, WLSE], F32, name="s23")
        nc.vector.tensor_add(out=s01[:], in0=ex[0][:], in1=ex[1][:])
        nc.vector.tensor_add(out=s23[:], in0=ex[2][:], in1=ex[3][:])
        nc.vector.tensor_add(out=s01[:], in0=s01[:], in1=s23[:])
        lse_res = lse_p.tile([NE, WLSE], F32, name="lse_res")
        nc.scalar.activation(out=lse_res[:], in_=s01[:], func=mybir.ActivationFunctionType.Ln)
        nc.scalar.dma_start(out=eemb_out[:], in_=lse_res[:])

        # --- sequence chunks (already bf16) + ones column ---
        seqb = []
        for k in range(NCH):
            sb = seqb_p.tile([128, HS + 1], BF16, name=f"sb{k}")
            nc.sync.dma_start(out=sb[:, 0:HS], in_=seq_b[k * 128 : (k + 1) * 128, :])
            nc.vector.memset(sb[:, HS : HS + 1], 1.0)
            seqb.append(sb)

        # --- per c-chunk: EA transpose-mean, pair products, h-reduction ---
        gs = []
        for k in range(NCH):
            # EA: PSUM [128 c, (h-major) 12*42], 24 tiny matmuls
            ps = ea_ps.tile([128, H * NE], F32, name="eaps")
            for t in range(2):
                for h in range(H):
                    nc.tensor.matmul(
                        out=ps[:, h * NE + t * 21 : h * NE + t * 21 + 21],
                        lhsT=raws[t][:, h * CH + k * 128 : h * CH + (k + 1) * 128],
                        rhs=sel_sb[:],
                        start=True,
                        stop=True,
                    )
            # drain + un-interleave to h-minor bf16 [128, (n, h)]
            ea = ea_p.tile([128, NE * H], BF16, name=f"ea{k}")
            nc.scalar.activation(
                out=ea[:].rearrange("p (n h) -> p n h", h=H),
                in_=ps[:].rearrange("p (h n) -> p n h", n=NE),
                func=mybir.ActivationFunctionType.Copy,
            )

            # pair products (DVE bf16 2x): pr[p, (u, h)]
            ea3 = ea[:].rearrange("p (n h) -> p n h", h=H)
            pr = pr_p.tile([128, U * H], BF16, name="pr")
            for b in range(NB):
                w = BLKW[b]
                jf = BH * b
                in0 = ea3[:, jf : jf + BH, :].unsqueeze(2).to_broadcast([128, BH, w, H])
                in1 = ea3[:, jf:NE, :].unsqueeze(1).to_broadcast([128, BH, w, H])
                sec = pr[:, BLKOFF[b] * H : (BLKOFF[b] + BH * w) * H]
                pr4 = sec.rearrange("p (i j h) -> p i j h", j=w, h=H)
                nc.vector.tensor_tensor(out=pr4, in0=in0, in1=in1, op=mybir.AluOpType.mult)

            # L1: 12 -> 6 on DVE (2x, aligned)
            pru = pr[:].rearrange("p (u h) -> p u h", h=H)
            t6 = t6_p.tile([128, U * 6], BF16, name="t6")
            t6v = t6[:].rearrange("p (u s) -> p u s", s=6)
            nc.vector.tensor_tensor(out=t6v, in0=pru[:, :, 0:6], in1=pru[:, :, 6:12], op=mybir.AluOpType.add)

            # L2: 6 -> 2 (two adds, 4B-aligned) ; L3: 2 -> 1
            eng = nc.gpsimd if TREE_GPSIMD else nc.vector
            x2 = x2_p.tile([128, U * 2], BF16, name="x2")
            x2v = x2[:].rearrange("p (u s) -> p u s", s=2)
            eng.tensor_tensor(out=x2v, in0=t6v[:, :, 0:2], in1=t6v[:, :, 2:4], op=mybir.AluOpType.add)
            eng.tensor_tensor(out=x2v, in0=x2v, in1=t6v[:, :, 4:6], op=mybir.AluOpType.add)
            gsum = gs_p.tile([128, U], BF16, name="gsum")
            a = x2v[:, :, 0:1].squeeze(2)
            bb = x2v[:, :, 1:2].squeeze(2)
            eng.tensor_tensor(out=gsum[:], in0=a, in1=bb, op=mybir.AluOpType.add)

            # relu on ACT; pad to 1024 rows so rs taus are 128 wide (FWL)
            g_t = g_p.tile([128, NTAU_P * UTAU], BF16, name=f"g{k}")
            nc.gpsimd.memset(g_t[:, U:], 0.0)
            nc.scalar.activation(out=g_t[:, 0:U], in_=gsum[:], func=mybir.ActivationFunctionType.Relu)
            gs.append(g_t)

        # --- rs matmuls: taus grouped to fit PSUM banks; accumulate over k ---
        # per-chunk rs partials: no cross-chunk PSUM liveness, so every tau's
        # matmuls run right after G_k lands and overlap the next chunk's DVE
        for k in range(NCH):
            for tau in range(NTAU_P):
                lo = tau * UTAU
                psA = rsA_ps.tile([UTAU, 512], F32, name="psA")
                psB = rsB_ps.tile([UTAU, HS + 1 - 512], F32, name="psB")
                nc.tensor.matmul(
                    out=psA[:], lhsT=gs[k][:, lo : lo + UTAU],
                    rhs=seqb[k][:, 0:512], start=True, stop=True,
                )
                nc.tensor.matmul(
                    out=psB[:], lhsT=gs[k][:, lo : lo + UTAU],
                    rhs=seqb[k][:, 512 : HS + 1], start=True, stop=True,
                )
                st = st_p.tile([UTAU, HS + 1], BF16, name="st")
                nc.scalar.activation(
                    out=st[:, 0:512], in_=psA[:],
                    func=mybir.ActivationFunctionType.Copy,
                )
                nc.scalar.activation(
                    out=st[:, 512 : HS + 1], in_=psB[:],
                    func=mybir.ActivationFunctionType.Copy,
                )
                glob = (k * NTAU_P + tau) * UTAU
                nc.sync.dma_start(out=rs_out[glob : glob + UTAU, :], in_=st[:])

    nc.finalize()
    return nc


def _host_inputs(sequence_output, attention, entity_starts):
    """Build the 8 per-core input maps."""
    sel_np = np.zeros([GT, NE // 2], np.float32)
    sel_np[np.arange(GT), np.arange(GT) // M] = 0.25
    sel_np = sel_np.astype(NP_BF16)

    in_maps = []
    for d in range(B):
        starts_doc = np.asarray(entity_starts[d], dtype=np.int64)
        pos = (starts_doc + OFFSET).astype(np.int32)      # [42, 4], < 1024

        ig = np.zeros([GT, 2], np.int32)
        for t in range(2):
            ig[:, t] = pos[21 * t + np.arange(GT) // M, np.arange(GT) % M]

        att_bf = np.asarray(attention[d], dtype=np.float32).astype(NP_BF16)  # [12,1024,1024]
        att_t = att_bf.transpose(1, 0, 2)                 # [pos, h, c]
        seq_doc = np.asarray(sequence_output[d], dtype=np.float32)

        for ch in range(2):
            csl = slice(ch * CH, (ch + 1) * CH)
            att_half = np.ascontiguousarray(att_t[:, :, csl])   # [1024, 12, 512]
            in_maps.append(
                {
                    "att": att_half.reshape(C, H * CH),
                    "seq_b": np.ascontiguousarray(seq_doc[csl, :]).astype(NP_BF16),
                    "seq_lse": np.ascontiguousarray(
                        seq_doc[:, ch * WLSE : (ch + 1) * WLSE]
                    ),
                    "sel": sel_np,
                    "idx_g": ig,
                    "idx_lse": pos,
                }
            )
    return in_maps


_row_table_cache = {}


def _grid_row_table():
    """[42, 42] -> packed canonical row (use at [min, max])."""
    if "t" not in _row_table_cache:
        row_of = np.full((NE, NE), -1, np.int64)
        for bb in range(NB):
            w = BLKW[bb]
            jf = BH * bb
            for il in range(BH):
                for j in range(jf, NE):
                    row_of[jf + il, j] = BLKOFF[bb] + il * w + (j - jf)
        _row_table_cache["t"] = row_of
    return _row_table_cache["t"]


def _assemble(results, hts):
    eemb = np.empty([B, NE, HS], np.float32)
    rs_rows = np.empty([B, U, HS], np.float32)
    row_of = _grid_row_table()
    for d in range(B):
        o0 = results[2 * d]["eemb_out"]
        o1 = results[2 * d + 1]["eemb_out"]
        eemb[d, :, 0:WLSE] = o0
        eemb[d, :, WLSE:HS] = o1

        p0 = np.asarray(results[2 * d]["rs_out"], dtype=np.float32)
        p1 = np.asarray(results[2 * d + 1]["rs_out"], dtype=np.float32)
        s = (p0 + p1).reshape(NCH, NTAU_P * UTAU, HS + 1).sum(axis=0)[:U]
        rs_rows[d] = s[:, 0:HS] / (s[:, HS : HS + 1] + 1e-10)

    hts_np = np.asarray(hts, dtype=np.int64)
    h_idx = hts_np[:, :, 0]                            # [B, 1764]
    t_idx = hts_np[:, :, 1]
    mn = np.minimum(h_idx, t_idx)
    mx = np.maximum(h_idx, t_idx)
    shape = (B, NE, NE, HS)
    hss = np.empty([B, NE * NE, HS], np.float32)
    rss = np.empty([B, NE * NE, HS], np.float32)
    tss = np.empty([B, NE * NE, HS], np.float32)
    for d in range(B):
        hss[d] = eemb[d][h_idx[d]]
        tss[d] = eemb[d][t_idx[d]]
        rss[d] = rs_rows[d][row_of[mn[d], mx[d]]]
    return hss.reshape(shape), rss.reshape(shape), tss.reshape(shape)


def kernel(sequence_output, attention, entity_starts, hts):
    if "nc" not in _prog_cache:
        _prog_cache["nc"] = _build_program()
    nc = _prog_cache["nc"]

    in_maps = _host_inputs(sequence_output, attention, entity_starts)
    res = run_bass_kernel_spmd(nc, in_maps, list(range(N_CORES))).results
    return _assemble(res, hts)


if __name__ == "__main__":
    # smoke test with random data
    rng = np.random.default_rng(0)
    seq = rng.standard_normal((B, C, HS), dtype=np.float32)
    att = rng.random((B, H, C, C), dtype=np.float32)
    starts = rng.integers(0, 1020, (B, NE, M))
    hts_a = rng.integers(0, NE, (B, NE * NE, 2))
    outs = kernel(seq, att, starts, hts_a)
    print([o.shape for o in outs])


# revision 29
# speedup vs baseline: 1.6184x; 1.0181x over previous
"""Trainium2 Bass kernel for DocREModel_KD head (ragged_sequence).

Problem shape (hardcoded, per spec):
  sequence_output [4, 1024, 768] f32
  attention       [4, 12, 1024, 1024] f32
  entity_starts   [4, 42, 4] int
  hts             [4, 1764, 2] int
Outputs: (hss, rss, tss) each [4, 42, 42, 768] f32.

Strategy v2 (8 cores, SPMD single program, c-split + host reduce):
  - 2 cores per document, split by the attention column dim c (512 each).
    Each core gathers only its c-half of the mention attention rows (staged
    host-side as bf16 [pos, h, c-half], h-contiguous so one mention is one
    24KB/2 contiguous row), computes the full canonical pair grid G over its
    c-half (relu is elementwise-in-c after the h-sum, so each half is
    complete), and produces UNNORMALIZED partial rs plus a partial
    normalizer via a ones-column. The host adds the two halves and
    normalizes (the unshard step).
  - Canonical pair packing: 7 i-blocks of height 6, block b covers
    j in [6b, 42): U = 6*(42+36+30+24+18+12+6) = 1008 = 8 taus of 126.
  - EA (mention-mean of attention, c-partitioned) via per-(tile,head,chunk)
    PE matmuls against a tiny [84, 21] 0.25-selection matrix (mean +
    transpose in one step, bf16 weights -> fast LDWEIGHTS); the ACT drain
    un-interleaves h-major PSUM bands to the h-minor layout the DVE
    pair-products need for 2x mode.
  - Pair products on DVE (bf16 2x) + 12->6 first reduction level on DVE;
    lower tree levels + final add on GpSimd; relu on ScalarE.
  - rs = relu(G) @ [seq | ones] accumulated over the 4 c-chunks in PSUM,
    taus grouped 3/3/2 to fit banks; drains to bf16 and DMA out.
  - e_emb logsumexp d-split across the core pair (exact fp32), as before.
  - hss/tss and the hts->grid mapping assembled host-side.
"""

import numpy as np
from contextlib import ExitStack

import concourse.bass as bass
import concourse.bacc as bacc
import concourse.mybir as mybir
import concourse.tile as tile
from concourse.bass_utils import run_bass_kernel_spmd

# ---- problem constants ----
B, H, C, HS, NE, M = 4, 12, 1024, 768, 42, 4
OFFSET = 1
CH = C // 2          # 512: c-half per core
NCH = CH // 128      # 4 c-chunks per core
BH = 6               # i-block height
NB = NE // BH        # 7 blocks
BLKW = [NE - BH * b for b in range(NB)]            # 42,36,30,24,18,12,6
BLKOFF = [BH * sum(BLKW[:b]) for b in range(NB)]   # packed row offsets
U = BH * sum(BLKW)   # 1008 packed canonical pair rows
TAU = 126
NTAU = U // TAU      # 8
GT = 84              # mentions per gather tile (21 entities x 4)
HH = H // 2          # 6 heads per gather half
WLSE = HS // 2       # 384: e_emb d-split width per core
N_CORES = 8

F32 = mybir.dt.float32
BF16 = mybir.dt.bfloat16
I32 = mybir.dt.int32
NP_BF16 = mybir.dt.np(BF16)

# tree lower levels on gpsimd (measured: gpsimd tensor_tensor is ~0.4 elem/cyc
# and contends with DVE on SBUF ports — keep False)
TREE_GPSIMD = False
UTAU = 128           # padded tau width (G padded to 1024 rows for FWL)
NTAU_P = 8

_prog_cache = {}


def _build_program():
    nc = bacc.Bacc(None)

    # att halves: [pos, (h, c)] rows; h-half hh covers heads 6*hh..6*hh+5
    att = nc.dram_tensor("att", [C, H * CH], BF16, kind="ExternalInput")
    seq_b = nc.dram_tensor("seq_b", [CH, HS], BF16, kind="ExternalInput")
    seq_lse = nc.dram_tensor("seq_lse", [C, WLSE], F32, kind="ExternalInput")
    sel_d = nc.dram_tensor("sel", [GT, NE // 2], BF16, kind="ExternalInput")
    idx_g_d = nc.dram_tensor("idx_g", [GT, 2], I32, kind="ExternalInput")
    idx_lse_d = nc.dram_tensor("idx_lse", [NE, M], I32, kind="ExternalInput")

    # per-chunk-pair unnormalized rs partials (+ ones-column), host-reduced
    rs_out = nc.dram_tensor(
        "rs_out", [(NCH // 2) * NTAU_P * UTAU, HS + 1], BF16, kind="ExternalOutput"
    )
    eemb_out = nc.dram_tensor("eemb_out", [NE, WLSE], F32, kind="ExternalOutput")

    with tile.TileContext(nc) as tc, ExitStack() as ctx:
        const_p = ctx.enter_context(tc.tile_pool(name="const", bufs=1))
        raw_p = ctx.enter_context(tc.tile_pool(name="raw", bufs=1))
        ea_p = ctx.enter_context(tc.tile_pool(name="ea", bufs=2))
        pr_p = ctx.enter_context(tc.tile_pool(name="pr", bufs=2))
        t6_p = ctx.enter_context(tc.tile_pool(name="t6", bufs=2))
        x2_p = ctx.enter_context(tc.tile_pool(name="x2", bufs=2))
        gs_p = ctx.enter_context(tc.tile_pool(name="gs", bufs=2))
        g_p = ctx.enter_context(tc.tile_pool(name="g", bufs=4))
        seqb_p = ctx.enter_context(tc.tile_pool(name="seqb", bufs=1))
        lse_p = ctx.enter_context(tc.tile_pool(name="lse", bufs=1))
        st_p = ctx.enter_context(tc.tile_pool(name="st", bufs=3))

        ea_ps = ctx.enter_context(tc.tile_pool(name="eaps", bufs=2, space="PSUM"))
        rsA_ps = ctx.enter_context(tc.tile_pool(name="rsA", bufs=3, space="PSUM"))
        rsB_ps = ctx.enter_context(tc.tile_pool(name="rsB", bufs=3, space="PSUM"))

        # --- constants / indices to SBUF ---
        ig_sb = const_p.tile([GT, 2], I32, name="ig_sb")
        nc.sync.dma_start(out=ig_sb[:], in_=idx_g_d[:])
        il_sb = const_p.tile([NE, M], I32, name="il_sb")
        nc.sync.dma_start(out=il_sb[:], in_=idx_lse_d[:])
        sel_sb = const_p.tile([GT, NE // 2], BF16, name="sel_sb")
        nc.sync.dma_start(out=sel_sb[:], in_=sel_d[:])

        # --- indirect gathers: attention mention rows (c-half, bf16) ---
        # raws[t]: [84 mentions, 12 heads * 512 c] (12KB contiguous rows)
        raws = []
        for t in range(2):
            rt = raw_p.tile([GT, H * CH], BF16, name=f"raw{t}")
            nc.gpsimd.indirect_dma_start(
                out=rt[:],
                out_offset=None,
                in_=att[:],
                in_offset=bass.IndirectOffsetOnAxis(ap=ig_sb[:, t : t + 1], axis=0),
            )
            raws.append(rt)

        # --- PE warm-up: dummy matmuls during the gather wait keep the HAM
        # clock gate open so the EA/rs matmuls run at full clock ---
        wz = const_p.tile([128, 128], BF16, name="wz")
        nc.vector.memset(wz[:], 0.0)
        wps = ea_ps.tile([128, H * NE], F32, name="eaps")
        for _ in range(32):
            nc.tensor.matmul(
                out=wps[:, 0:128], lhsT=wz[:], rhs=wz[:], start=True, stop=True,
            )

        # --- e_emb logsumexp pipeline (d-split half, exact fp32) ---
        sg = []
        for r in range(M):
            g = lse_p.tile([NE, WLSE], F32, name=f"sg{r}")
            nc.gpsimd.indirect_dma_start(
                out=g[:],
                out_offset=None,
                in_=seq_lse[:],
                in_offset=bass.IndirectOffsetOnAxis(ap=il_sb[:, r : r + 1], axis=0),
            )
            sg.append(g)
        ex = []
        for r in range(M):
            e = lse_p.tile([NE, WLSE], F32, name=f"ex{r}")
            nc.scalar.activation(out=e[:], in_=sg[r][:], func=mybir.ActivationFunctionType.Exp)
            ex.append(e)
        s01 = lse_p.tile([NE, WLSE], F32, name="s01")
        s23 = lse_p.tile([NE, WLSE], F32, name="s23")
        nc.vector.tensor_add(out=s01[:], in0=ex[0][:], in1=ex[1][:])
        nc.vector.tensor_add(out=s23[:], in0=ex[2][:], in1=ex[3][:])
        nc.vector.tensor_add(out=s01[:], in0=s01[:], in1=s23[:])
        lse_res = lse_p.tile([NE, WLSE], F32, name="lse_res")
        nc.scalar.activation(out=lse_res[:], in_=s01[:], func=mybir.ActivationFunctionType.Ln)
        nc.scalar.dma_start(out=eemb_out[:], in_=lse_res[:])

        # --- sequence chunks (already bf16) + ones column ---
        seqb = []
        for k in range(NCH):
            sb = seqb_p.tile([128, HS + 1], BF16, name=f"sb{k}")
            nc.sync.dma_start(out=sb[:, 0:HS], in_=seq_b[k * 128 : (k + 1) * 128, :])
            nc.vector.memset(sb[:, HS : HS + 1], 1.0)
            seqb.append(sb)

        # --- per c-chunk: EA transpose-mean, pair products, h-reduction ---
        gs = []
        for k in range(NCH):
            # EA: PSUM [128 c, (h-major) 12*42], 24 tiny matmuls
            ps = ea_ps.tile([128, H * NE], F32, name="eaps")
            for t in range(2):
                for h in range(H):
                    nc.tensor.matmul(
                        out=ps[:, h * NE + t * 21 : h * NE + t * 21 + 21],
                        lhsT=raws[t][:, h * CH + k * 128 : h * CH + (k + 1) * 128],
                        rhs=sel_sb[:],
                        start=True,
                        stop=True,
                    )
            # drain + un-interleave to h-minor bf16 [128, (n, h)]
            ea = ea_p.tile([128, NE * H], BF16, name=f"ea{k}")
            nc.scalar.activation(
                out=ea[:].rearrange("p (n h) -> p n h", h=H),
                in_=ps[:].rearrange("p (h n) -> p n h", n=NE),
                func=mybir.ActivationFunctionType.Copy,
            )

            # pair products (DVE bf16 2x): pr[p, (u, h)]
            ea3 = ea[:].rearrange("p (n h) -> p n h", h=H)
            pr = pr_p.tile([128, U * H], BF16, name="pr")
            for b in range(NB):
                w = BLKW[b]
                jf = BH * b
                in0 = ea3[:, jf : jf + BH, :].unsqueeze(2).to_broadcast([128, BH, w, H])
                in1 = ea3[:, jf:NE, :].unsqueeze(1).to_broadcast([128, BH, w, H])
                sec = pr[:, BLKOFF[b] * H : (BLKOFF[b] + BH * w) * H]
                pr4 = sec.rearrange("p (i j h) -> p i j h", j=w, h=H)
                nc.vector.tensor_tensor(out=pr4, in0=in0, in1=in1, op=mybir.AluOpType.mult)

            # L1: 12 -> 6 on DVE (2x, aligned)
            pru = pr[:].rearrange("p (u h) -> p u h", h=H)
            t6 = t6_p.tile([128, U * 6], BF16, name="t6")
            t6v = t6[:].rearrange("p (u s) -> p u s", s=6)
            nc.vector.tensor_tensor(out=t6v, in0=pru[:, :, 0:6], in1=pru[:, :, 6:12], op=mybir.AluOpType.add)

            # L2: 6 -> 2 (two adds, 4B-aligned) ; L3: 2 -> 1
            eng = nc.gpsimd if TREE_GPSIMD else nc.vector
            x2 = x2_p.tile([128, U * 2], BF16, name="x2")
            x2v = x2[:].rearrange("p (u s) -> p u s", s=2)
            eng.tensor_tensor(out=x2v, in0=t6v[:, :, 0:2], in1=t6v[:, :, 2:4], op=mybir.AluOpType.add)
            eng.tensor_tensor(out=x2v, in0=x2v, in1=t6v[:, :, 4:6], op=mybir.AluOpType.add)
            gsum = gs_p.tile([128, U], BF16, name="gsum")
            a = x2v[:, :, 0:1].squeeze(2)
            bb = x2v[:, :, 1:2].squeeze(2)
            eng.tensor_tensor(out=gsum[:], in0=a, in1=bb, op=mybir.AluOpType.add)

            # relu on ACT; pad to 1024 rows so rs taus are 128 wide (FWL)
            g_t = g_p.tile([128, NTAU_P * UTAU], BF16, name=f"g{k}")
            nc.gpsimd.memset(g_t[:, U:], 0.0)
            nc.scalar.activation(out=g_t[:, 0:U], in_=gsum[:], func=mybir.ActivationFunctionType.Relu)
            gs.append(g_t)

        # --- rs matmuls: taus grouped to fit PSUM banks; accumulate over k ---
        # rs partials accumulated over chunk PAIRS (k01, k23): short PSUM
        # liveness (taus rotate through 3 bank-pairs and overlap the DVE
        # pipeline) at half the drain volume of per-chunk partials
        for kp in range(NCH // 2):
            if kp > 0:
                # re-warm the PE while it waits for the last chunks' G
                wps2 = ea_ps.tile([128, H * NE], F32, name="eaps")
                for _ in range(24):
                    nc.tensor.matmul(
                        out=wps2[:, 0:128], lhsT=wz[:], rhs=wz[:], start=True, stop=True,
                    )
            for tau in range(NTAU_P):
                lo = tau * UTAU
                psA = rsA_ps.tile([UTAU, 512], F32, name="psA")
                psB = rsB_ps.tile([UTAU, HS + 1 - 512], F32, name="psB")
                for k in (2 * kp, 2 * kp + 1):
                    nc.tensor.matmul(
                        out=psA[:], lhsT=gs[k][:, lo : lo + UTAU],
                        rhs=seqb[k][:, 0:512], start=(k == 2 * kp), stop=(k == 2 * kp + 1),
                    )
                    nc.tensor.matmul(
                        out=psB[:], lhsT=gs[k][:, lo : lo + UTAU],
                        rhs=seqb[k][:, 512 : HS + 1], start=(k == 2 * kp), stop=(k == 2 * kp + 1),
                    )
                st = st_p.tile([UTAU, HS + 1], BF16, name="st")
                nc.scalar.activation(
                    out=st[:, 0:512], in_=psA[:],
                    func=mybir.ActivationFunctionType.Copy,
                )
                nc.scalar.activation(
                    out=st[:, 512 : HS + 1], in_=psB[:],
                    func=mybir.ActivationFunctionType.Copy,
                )
                glob = (kp * NTAU_P + tau) * UTAU
                nc.sync.dma_start(out=rs_out[glob : glob + UTAU, :], in_=st[:])

    nc.finalize()
    return nc


def _host_inputs(sequence_output, attention, entity_starts):
    """Build the 8 per-core input maps."""
    sel_np = np.zeros([GT, NE // 2], np.float32)
    sel_np[np.arange(GT), np.arange(GT) // M] = 0.25
    sel_np = sel_np.astype(NP_BF16)

    in_maps = []
    for d in range(B):
        starts_doc = np.asarray(entity_starts[d], dtype=np.int64)
        pos = (starts_doc + OFFSET).astype(np.int32)      # [42, 4], < 1024

        ig = np.zeros([GT, 2], np.int32)
        for t in range(2):
            ig[:, t] = pos[21 * t + np.arange(GT) // M, np.arange(GT) % M]

        att_bf = np.asarray(attention[d], dtype=np.float32).astype(NP_BF16)  # [12,1024,1024]
        att_t = att_bf.transpose(1, 0, 2)                 # [pos, h, c]
        seq_doc = np.asarray(sequence_output[d], dtype=np.float32)

        for ch in range(2):
            csl = slice(ch * CH, (ch + 1) * CH)
            att_half = np.ascontiguousarray(att_t[:, :, csl])   # [1024, 12, 512]
            in_maps.append(
                {
                    "att": att_half.reshape(C, H * CH),
                    "seq_b": np.ascontiguousarray(seq_doc[csl, :]).astype(NP_BF16),
                    "seq_lse": np.ascontiguousarray(
                        seq_doc[:, ch * WLSE : (ch + 1) * WLSE]
                    ),
                    "sel": sel_np,
                    "idx_g": ig,
                    "idx_lse": pos,
                }
            )
    return in_maps


_row_table_cache = {}


def _grid_row_table():
    """[42, 42] -> packed canonical row (use at [min, max])."""
    if "t" not in _row_table_cache:
        row_of = np.full((NE, NE), -1, np.int64)
        for bb in range(NB):
            w = BLKW[bb]
            jf = BH * bb
            for il in range(BH):
                for j in range(jf, NE):
                    row_of[jf + il, j] = BLKOFF[bb] + il * w + (j - jf)
        _row_table_cache["t"] = row_of
    return _row_table_cache["t"]


def _assemble(results, hts):
    eemb = np.empty([B, NE, HS], np.float32)
    rs_rows = np.empty([B, U, HS], np.float32)
    row_of = _grid_row_table()
    for d in range(B):
        o0 = results[2 * d]["eemb_out"]
        o1 = results[2 * d + 1]["eemb_out"]
        eemb[d, :, 0:WLSE] = o0
        eemb[d, :, WLSE:HS] = o1

        p0 = np.asarray(results[2 * d]["rs_out"], dtype=np.float32)
        p1 = np.asarray(results[2 * d + 1]["rs_out"], dtype=np.float32)
        s = (p0 + p1).reshape(NCH // 2, NTAU_P * UTAU, HS + 1).sum(axis=0)[:U]
        rs_rows[d] = s[:, 0:HS] / (s[:, HS : HS + 1] + 1e-10)

    hts_np = np.asarray(hts, dtype=np.int64)
    h_idx = hts_np[:, :, 0]                            # [B, 1764]
    t_idx = hts_np[:, :, 1]
    mn = np.minimum(h_idx, t_idx)
    mx = np.maximum(h_idx, t_idx)
    shape = (B, NE, NE, HS)
    hss = np.empty([B, NE * NE, HS], np.float32)
    rss = np.empty([B, NE * NE, HS], np.float32)
    tss = np.empty([B, NE * NE, HS], np.float32)
    for d in range(B):
        hss[d] = eemb[d][h_idx[d]]
        tss[d] = eemb[d][t_idx[d]]
        rss[d] = rs_rows[d][row_of[mn[d], mx[d]]]
    return hss.reshape(shape), rss.reshape(shape), tss.reshape(shape)


def kernel(sequence_output, attention, entity_starts, hts):
    if "nc" not in _prog_cache:
        _prog_cache["nc"] = _build_program()
    nc = _prog_cache["nc"]

    in_maps = _host_inputs(sequence_output, attention, entity_starts)
    res = run_bass_kernel_spmd(nc, in_maps, list(range(N_CORES))).results
    return _assemble(res, hts)


if __name__ == "__main__":
    # smoke test with random data
    rng = np.random.default_rng(0)
    seq = rng.standard_normal((B, C, HS), dtype=np.float32)
    att = rng.random((B, H, C, C), dtype=np.float32)
    starts = rng.integers(0, 1020, (B, NE, M))
    hts_a = rng.integers(0, NE, (B, NE * NE, 2))
    outs = kernel(seq, att, starts, hts_a)
    print([o.shape for o in outs])
